# revision 1
# baseline (speedup 1.0000x reference)
"""Depthwise causal Conv1d (B=4, S=4096, D=2048, K=4) on 8 TRN2 NeuronCores.

Sharding: 8 cores = batch(4) x sequence-halves(2); zero communication.
Each core receives a channel-major slab x_core[D, 3 + S/2] (3 history
columns: zeros at sequence start, else the previous half's tail), computes

    out[d, s] = sum_k w[d, k] * x[d, s - 3 + k] + bias[d]

with per-128-channel-block ops (free dim = 2048 sequence positions)
spread over three engines (walrus only allows per-partition-scalar ops
on DVE and ACT; POOL gets the plain tensor add):

    m3 = x3 * w3 + bias         (ACT  activation, scale+bias APs)
    m2 = x2 * w2                (ACT  activation, scale AP)
    s  = m3 + m2                (POOL tensor_tensor add)
    b  = x1 * w1 + s            (DVE  scalar_tensor_tensor)
    o  = x0 * w0 + b            (DVE  scalar_tensor_tensor -> out tile)

All DMAs are contiguous ~1 MB slabs over 128 partitions; inputs ride the
SP HWDGE ring, outputs the ACT ring, so neither blocks the other.
"""

import numpy as np

import concourse.bacc as bacc
import concourse.mybir as mybir
from concourse.bass_utils import run_bass_kernel_spmd
from concourse.tile import TileContext

B, S, D, K = 4, 4096, 2048, 4
NCORES = 8
SHALF = S // 2          # 2048 sequence positions per core
HIST = K - 1            # 3 history columns
NBLK = D // 128         # 16 channel blocks
F32 = mybir.dt.float32
MULT = mybir.AluOpType.mult
ADD = mybir.AluOpType.add

_CACHE = {}


def _build_program(nreps=1):
    key = ("nc", nreps)
    if key in _CACHE:
        return _CACHE[key]
    nc = bacc.Bacc("TRN2", num_devices=NCORES)
    x_d = nc.dram_tensor("xin", [D, SHALF + HIST], F32, kind="ExternalInput").ap()
    # wtab[p, k*NBLK+blk] = w[blk*128+p, k] for k<4; wtab[p, 4*NBLK+blk] = bias
    w_d = nc.dram_tensor("wtab", [128, (K + 1) * NBLK], F32, kind="ExternalInput").ap()
    o_d = nc.dram_tensor("out", [D, SHALF], F32, kind="ExternalOutput").ap()

    with TileContext(nc) as tc:
        with (
            tc.tile_pool(name="const", bufs=1) as const,
            tc.tile_pool(name="xpool", bufs=6) as xpool,
            tc.tile_pool(name="m3pool", bufs=4) as m3pool,
            tc.tile_pool(name="m2pool", bufs=4) as m2pool,
            tc.tile_pool(name="spool", bufs=4) as spool,
            tc.tile_pool(name="opool", bufs=5) as opool,
        ):
            wsb = const.tile([128, (K + 1) * NBLK], F32, tag="wsb")
            # weight table rides the ACT ring so x block 0 starts immediately
            nc.scalar.dma_start(out=wsb[:], in_=w_d)

            def wcol(k, blk):
                return wsb[:, k * NBLK + blk : k * NBLK + blk + 1]

            # out-DMAs are issued OUT_DELAY blocks late so the ACT sequencer
            # never stalls waiting for a chain result before its next
            # activation op (software-pipelined DMA issue)
            OUT_DELAY = 2
            pending = []

            def flush_out(upto):
                while pending and pending[0][0] <= upto:
                    i, tile_ap = pending.pop(0)
                    i %= NBLK
                    nc.scalar.dma_start(
                        out=o_d[i * 128 : (i + 1) * 128, :], in_=tile_ap
                    )

            for blk_r in range(NBLK * nreps):
                blk = blk_r % NBLK
                xt = xpool.tile([128, SHALF + HIST], F32, tag="xt")
                nc.sync.dma_start(
                    out=xt[:], in_=x_d[blk * 128 : (blk + 1) * 128, :]
                )
                ot = opool.tile([128, SHALF], F32, tag="ot")

                # first/last blocks run as 4 short sub-chains so the pipeline
                # fills (first out-DMA ready early) and drains (short tail)
                # quickly; middle blocks use one full-width chain
                edge = blk_r == 0 or blk_r == NBLK * nreps - 1
                FD = SHALF // 4 if edge else SHALF
                for h in range(SHALF // FD):
                    lo = h * FD

                    def tap(k):
                        return xt[:, lo + k : lo + k + FD]

                    osl = ot[:, lo : lo + FD]
                    m3 = m3pool.tile([128, FD], F32, tag="m3", name=f"m3_{blk_r}_{h}")
                    nc.scalar.activation(
                        m3[:],
                        tap(3),
                        mybir.ActivationFunctionType.Identity,
                        bias=wcol(K, blk),
                        scale=wcol(3, blk),
                    )
                    m2 = m2pool.tile([128, FD], F32, tag="m2", name=f"m2_{blk_r}_{h}")
                    nc.scalar.activation(
                        m2[:],
                        tap(2),
                        mybir.ActivationFunctionType.Copy,
                        bias=0.0,
                        scale=wcol(2, blk),
                    )
                    s = spool.tile([128, FD], F32, tag="s", name=f"s_{blk_r}_{h}")
                    nc.gpsimd.tensor_tensor(out=s[:], in0=m3[:], in1=m2[:], op=ADD)
                    # b lands in ot, final stt is in-place (saves a pool)
                    nc.vector.scalar_tensor_tensor(
                        osl, tap(1), wcol(1, blk), s[:], MULT, ADD
                    )
                    nc.vector.scalar_tensor_tensor(
                        osl, tap(0), wcol(0, blk), osl, MULT, ADD
                    )
                pending.append((blk_r, ot[:]))
                flush_out(blk_r - OUT_DELAY)
            flush_out(NBLK * nreps)

    nc.compile()
    _CACHE["nc"] = nc
    return nc


def _shard_inputs(x, weight, bias):
    x = np.asarray(x, dtype=np.float32)
    weight = np.asarray(weight, dtype=np.float32)
    bias = np.asarray(bias, dtype=np.float32)

    wr = weight[:, 0, :].reshape(NBLK, 128, K)          # [blk, p, k]
    wtab = np.empty((128, (K + 1) * NBLK), dtype=np.float32)
    wtab[:, : K * NBLK] = wr.transpose(1, 2, 0).reshape(128, K * NBLK)
    wtab[:, K * NBLK :] = bias.reshape(NBLK, 128).T

    in_maps = []
    for core in range(NCORES):
        b, h = divmod(core, 2)
        s0 = h * SHALF
        xc = np.empty((D, SHALF + HIST), dtype=np.float32)
        xbt = x[b].T  # [D, S] view
        if s0 == 0:
            xc[:, :HIST] = 0.0
            xc[:, HIST:] = xbt[:, :SHALF]
        else:
            xc[:] = xbt[:, s0 - HIST : s0 + SHALF]
        in_maps.append({"xin": xc, "wtab": wtab})
    return in_maps


def _run(x, weight, bias, trace=False):
    nc = _build_program()
    in_maps = _shard_inputs(x, weight, bias)
    res = run_bass_kernel_spmd(nc, in_maps, list(range(NCORES)), trace=trace)
    out = np.empty((B, S, D), dtype=np.float32)
    for core in range(NCORES):
        b, h = divmod(core, 2)
        out[b, h * SHALF : (h + 1) * SHALF, :] = res.results[core]["out"].T
    return out, res


def kernel(x, weight, bias):
    out, _ = _run(x, weight, bias, trace=False)
    return out



# revision 2
# speedup vs baseline: 5.4311x; 5.4311x over previous
"""Depthwise causal Conv1d (B=4, S=4096, D=2048, K=4) on 8 TRN2 NeuronCores.

v2: bf16 staging + linearized DRAM layout + PE diag-matmul offload.

Sharding: 8 cores = batch(4) x sequence-halves(2); zero communication.
Host stages per-core inputs as bf16 in a channel-block-linearized layout

    xin[p, blk*2051 + c] = x[b, s0 - 3 + c, blk*128 + p]      (zeros for s<0)

so one DMA instruction per 4-block group moves 2.1 MB with 16.4 KB
contiguous descriptors.  Output is staged the same way (bf16) and
de-linearized + upcast on the host.

Per 128-channel block the conv out = sum_k w_k * shift_k(x) + bias is
computed by one of three engine pipelines, chosen per block to balance
engine occupancy (all tiles [128, 2048] bf16):

  PE blocks (10):  4 chunks x 4 taps of diag(w_k) matmuls accumulated in
                   PSUM f32, drained (+bias) to bf16 by ACT activation for
                   6 blocks and by DVE affine_then_add(zero) for 4 blocks
  ACT2 blocks (6): ACT t3 = w3*x3 + bias; ACT t2 = w2*x2; POOL s = t3+t2;
                   DVE affine_then_add w1*x1 + s; DVE stt w0*x0 + . -> out

(POOL cannot take per-partition scalar operands on TRN2 walrus, so all
channel-wise scaling lives on ACT/DVE/PE.)

Inputs ride the SP HWDGE ring, outputs the ACT ring (issued one group
late so the ACT sequencer never stalls on a not-yet-finished chain).
"""

import numpy as np

import concourse.bacc as bacc
import concourse.mybir as mybir
from concourse.bass_utils import run_bass_kernel_spmd
from concourse.tile import TileContext

B, S, D, K = 4, 4096, 2048, 4
NCORES = 8
SHALF = S // 2           # 2048 sequence positions per core
HIST = K - 1             # 3 history columns
CW = SHALF + HIST        # 2051 staged columns per block
NBLK = D // 128          # 16 channel blocks
GRP = 4                  # blocks per DMA group
NGRP = NBLK // GRP
CHUNK = 512              # PSUM bank = 512 f32
F32 = mybir.dt.float32
BF16 = mybir.dt.bfloat16
MULT = mybir.AluOpType.mult
ADD = mybir.AluOpType.add
IDENT = mybir.ActivationFunctionType.Identity

PE_BLKS = (0, 2, 3, 4, 6, 7, 8, 10, 11, 12, 14)
PE_DVE_DRAIN = (2, 6, 10, 12, 14)  # PE blocks whose PSUM drain rides DVE
# remaining blocks (1, 5, 9, 13, 15) take the ACT2 pipeline; types are
# interleaved within every DMA group so no engine sees a burst
PE_IDX = {b: i for i, b in enumerate(PE_BLKS)}

_CACHE = {}


def _build_program(nreps=1, grp=None, xbufs=4, obufs=4):
    global GRP, NGRP
    if grp is not None:
        GRP, NGRP = grp, NBLK // grp
    key = ("nc", nreps, GRP, xbufs, obufs)
    if key in _CACHE:
        return _CACHE[key]
    nc = bacc.Bacc("TRN2", num_devices=NCORES)
    x_d = nc.dram_tensor("xin", [128, NBLK * CW], BF16, kind="ExternalInput").ap()
    # wtab[p, k*NBLK+blk] = w[blk*128+p, k] for k<4; wtab[p, 4*NBLK+blk] = bias
    w_d = nc.dram_tensor("wtab", [128, (K + 1) * NBLK], F32, kind="ExternalInput").ap()
    # dwt[p, (pe_idx*K+k)*128 + q] = w[blk*128+p, k] if p == q else 0
    dw_d = nc.dram_tensor(
        "dwt", [128, len(PE_BLKS) * K * 128], BF16, kind="ExternalInput"
    ).ap()
    o_d = nc.dram_tensor("out", [128, NBLK * SHALF], BF16, kind="ExternalOutput").ap()

    with TileContext(nc) as tc:
        with (
            tc.tile_pool(name="const", bufs=1) as const,
            tc.tile_pool(name="xpool", bufs=xbufs) as xpool,
            tc.tile_pool(name="opool", bufs=obufs) as opool,
            tc.tile_pool(name="m3pool", bufs=2) as m3pool,
            tc.tile_pool(name="d2pool", bufs=2) as d2pool,
            tc.tile_pool(name="d1pool", bufs=2) as d1pool,
            tc.tile_pool(name="c2pool", bufs=2) as c2pool,
            tc.tile_pool(name="psum", bufs=8, space="PSUM") as psum,
        ):
            wsb = const.tile([128, (K + 1) * NBLK], F32, tag="wsb")
            nc.scalar.dma_start(out=wsb[:], in_=w_d)
            dwsb = const.tile([128, len(PE_BLKS) * K * 128], BF16, tag="dwsb")
            nc.scalar.dma_start(out=dwsb[:], in_=dw_d)
            zero = const.tile([128, CHUNK], BF16, tag="zero")
            nc.vector.memset(zero[:], 0.0)

            def wcol(k, blk):
                return wsb[:, k * NBLK + blk : k * NBLK + blk + 1]

            def dmat(pe_idx, k):
                c0 = (pe_idx * K + k) * 128
                return dwsb[:, c0 : c0 + 128]

            pending = []

            def flush_out(upto):
                while pending and pending[0][0] <= upto:
                    g, tile_ap = pending.pop(0)
                    g %= NGRP
                    nc.gpsimd.dma_start(
                        out=o_d[:, g * GRP * SHALF : (g + 1) * GRP * SHALF],
                        in_=tile_ap,
                    )

            xt = ot = None
            for blk_r in range(NBLK * nreps):
                blk = blk_r % NBLK
                g_r, j = divmod(blk_r, GRP)
                g = g_r % NGRP
                if j == 0:
                    flush_out(g_r - 2)
                    xt = xpool.tile([128, GRP * CW], BF16, tag="xt")
                    nc.sync.dma_start(
                        out=xt[:], in_=x_d[:, g * GRP * CW : (g + 1) * GRP * CW]
                    )
                    ot = opool.tile([128, GRP * SHALF], BF16, tag="ot")

                base = j * CW

                def tap(k, lo=0, n=SHALF):
                    return xt[:, base + k + lo : base + k + lo + n]

                osl = ot[:, j * SHALF : (j + 1) * SHALF]

                if blk in PE_IDX:
                    pi = PE_IDX[blk]
                    for c in range(SHALF // CHUNK):
                        ps = psum.tile([128, CHUNK], F32, tag="ps",
                                       name=f"ps_{blk_r}_{c}")
                        for k in range(K):
                            nc.tensor.matmul(
                                ps[:],
                                dmat(pi, k),
                                tap(k, c * CHUNK, CHUNK),
                                start=(k == 0),
                                stop=(k == K - 1),
                            )
                        oc = ot[:, j * SHALF + c * CHUNK : j * SHALF + (c + 1) * CHUNK]
                        if blk in PE_DVE_DRAIN:
                            nc.vector.affine_then_add(
                                oc, ps[:], zero[:], 1.0, wcol(K, blk)
                            )
                        else:
                            nc.scalar.activation(
                                oc, ps[:], IDENT, bias=wcol(K, blk), scale=1.0
                            )
                else:
                    t3 = m3pool.tile([128, SHALF], BF16, tag="t3",
                                     name=f"t3_{blk_r}")
                    nc.scalar.activation(
                        t3[:], tap(3), IDENT, bias=wcol(K, blk), scale=wcol(3, blk)
                    )
                    t2 = c2pool.tile([128, SHALF], BF16, tag="t2",
                                     name=f"t2_{blk_r}")
                    nc.scalar.activation(
                        t2[:], tap(2), IDENT, bias=0.0, scale=wcol(2, blk)
                    )
                    s = d2pool.tile([128, SHALF], BF16, tag="s",
                                    name=f"s_{blk_r}")
                    nc.gpsimd.tensor_tensor(out=s[:], in0=t3[:], in1=t2[:], op=ADD)
                    d1 = d1pool.tile([128, SHALF], BF16, tag="d1",
                                     name=f"d1_{blk_r}")
                    nc.vector.affine_then_add(d1[:], tap(1), s[:], wcol(1, blk), 0.0)
                    nc.vector.scalar_tensor_tensor(
                        osl, tap(0), wcol(0, blk), d1[:], MULT, ADD
                    )

                if j == GRP - 1:
                    pending.append((g_r, ot[:]))
            flush_out(NGRP * nreps)

    nc.compile()
    _CACHE[key] = nc
    return nc


def _shard_inputs(x, weight, bias):
    x = np.asarray(x, dtype=np.float32)
    weight = np.asarray(weight, dtype=np.float32)
    bias = np.asarray(bias, dtype=np.float32)

    wr = weight[:, 0, :].reshape(NBLK, 128, K)          # [blk, p, k]
    wtab = np.empty((128, (K + 1) * NBLK), dtype=np.float32)
    wtab[:, : K * NBLK] = wr.transpose(1, 2, 0).reshape(128, K * NBLK)
    wtab[:, K * NBLK :] = bias.reshape(NBLK, 128).T

    dwt = np.zeros((128, len(PE_BLKS) * K * 128), dtype=np.float32)
    rng = np.arange(128)
    for pi, blk in enumerate(PE_BLKS):
        for k in range(K):
            dwt[rng, (pi * K + k) * 128 + rng] = wr[blk, :, k]
    dwt = _to_bf16(dwt)

    in_maps = []
    for core in range(NCORES):
        b, h = divmod(core, 2)
        s0 = h * SHALF
        xc = np.empty((NBLK, 128, CW), dtype=np.float32)
        xbt = x[b].T.reshape(NBLK, 128, S)  # [blk, p, s] view
        if s0 == 0:
            xc[:, :, :HIST] = 0.0
            xc[:, :, HIST:] = xbt[:, :, :SHALF]
        else:
            xc[:] = xbt[:, :, s0 - HIST : s0 + SHALF]
        xin = np.ascontiguousarray(xc.transpose(1, 0, 2)).reshape(128, NBLK * CW)
        in_maps.append(
            {"xin": _to_bf16(xin), "wtab": wtab, "dwt": dwt}
        )
    return in_maps


def _bf16_dtype():
    import ml_dtypes

    return np.dtype(ml_dtypes.bfloat16)


def _to_bf16(a):
    return np.asarray(a, dtype=np.float32).astype(_bf16_dtype())


def _run(x, weight, bias, trace=False):
    nc = _build_program()
    in_maps = _shard_inputs(x, weight, bias)
    res = run_bass_kernel_spmd(nc, in_maps, list(range(NCORES)), trace=trace)
    out = np.empty((B, S, D), dtype=np.float32)
    for core in range(NCORES):
        b, h = divmod(core, 2)
        o = np.asarray(res.results[core]["out"]).astype(np.float32)
        o = o.reshape(128, NBLK, SHALF).transpose(1, 0, 2).reshape(D, SHALF)
        out[b, h * SHALF : (h + 1) * SHALF, :] = o.T
    return out, res


def kernel(x, weight, bias):
    out, _ = _run(x, weight, bias, trace=False)
    return out


# revision 3
# speedup vs baseline: 7.5455x; 1.3893x over previous
"""Depthwise causal Conv1d (B=4, S=4096, D=2048, K=4) on 8 TRN2 NeuronCores.

v2: bf16 staging + linearized DRAM layout + PE diag-matmul offload.

Sharding: 8 cores = batch(4) x sequence-halves(2); zero communication.
Host stages per-core inputs as bf16 in a channel-block-linearized layout

    xin[p, blk*2051 + c] = x[b, s0 - 3 + c, blk*128 + p]      (zeros for s<0)

so one DMA instruction per 4-block group moves 2.1 MB with 16.4 KB
contiguous descriptors.  Output is staged the same way (bf16) and
de-linearized + upcast on the host.

Per 128-channel block the conv out = sum_k w_k * shift_k(x) + bias is
computed by one of three engine pipelines, chosen per block to balance
engine occupancy (all tiles [128, 2048] bf16):

  PE blocks (11):  4 chunks x 4 taps of diag(w_k) matmuls accumulated in
                   PSUM f32, drained (+bias) to bf16 by ACT activation or
                   DVE affine_then_add(zero), alternating per chunk (24/20)
  ACT2 blocks (5): ACT t3 = w3*x3 + bias; ACT t2 = w2*x2; POOL s = t3+t2;
                   DVE affine_then_add w1*x1 + s; DVE stt w0*x0 + . -> out

(POOL cannot take per-partition scalar operands on TRN2 walrus, so all
channel-wise scaling lives on ACT/DVE/PE.)

Inputs ride the SP HWDGE ring, outputs the ACT ring (issued one group
late so the ACT sequencer never stalls on a not-yet-finished chain).
"""

import numpy as np

import concourse.bacc as bacc
import concourse.mybir as mybir
from concourse.bass_utils import run_bass_kernel_spmd
from concourse.tile import TileContext

B, S, D, K = 4, 4096, 2048, 4
NCORES = 8
SHALF = S // 2           # 2048 sequence positions per core
HIST = K - 1             # 3 history columns
CW = SHALF + HIST        # 2051 staged columns per block
NBLK = D // 128          # 16 channel blocks
GRP = 4                  # blocks per DMA group
NGRP = NBLK // GRP
CHUNK = 512              # PSUM bank = 512 f32
F32 = mybir.dt.float32
BF16 = mybir.dt.bfloat16
MULT = mybir.AluOpType.mult
ADD = mybir.AluOpType.add
IDENT = mybir.ActivationFunctionType.Identity

PE_BLKS = (0, 2, 3, 4, 6, 7, 8, 10, 11, 12, 14)
PE_DVE_DRAIN = (2, 6, 10, 12, 14)  # PE blocks whose PSUM drain rides DVE
# remaining blocks (1, 5, 9, 13, 15) take the ACT2 pipeline; types are
# interleaved within every DMA group so no engine sees a burst
PE_IDX = {b: i for i, b in enumerate(PE_BLKS)}

_CACHE = {}


def _build_program(nreps=1, grp=None, xbufs=6, obufs=4, chunk_drain=True):
    global GRP, NGRP
    if grp is not None:
        GRP, NGRP = grp, NBLK // grp
    key = ("nc", nreps, GRP, xbufs, obufs, chunk_drain)
    if key in _CACHE:
        return _CACHE[key]
    nc = bacc.Bacc("TRN2", num_devices=NCORES)
    x_d = nc.dram_tensor("xin", [128, NBLK * CW], BF16, kind="ExternalInput").ap()
    # wtab[p, k*NBLK+blk] = w[blk*128+p, k] for k<4; wtab[p, 4*NBLK+blk] = bias
    w_d = nc.dram_tensor("wtab", [128, (K + 1) * NBLK], F32, kind="ExternalInput").ap()
    # dwt[p, (pe_idx*K+k)*128 + q] = w[blk*128+p, k] if p == q else 0
    dw_d = nc.dram_tensor(
        "dwt", [128, len(PE_BLKS) * K * 128], BF16, kind="ExternalInput"
    ).ap()
    o_d = nc.dram_tensor("out", [128, NBLK * SHALF], BF16, kind="ExternalOutput").ap()

    with TileContext(nc) as tc:
        with (
            tc.tile_pool(name="const", bufs=1) as const,
            tc.tile_pool(name="xpool", bufs=xbufs) as xpool,
            tc.tile_pool(name="opool", bufs=obufs) as opool,
            tc.tile_pool(name="m3pool", bufs=2) as m3pool,
            tc.tile_pool(name="d2pool", bufs=2) as d2pool,
            tc.tile_pool(name="d1pool", bufs=2) as d1pool,
            tc.tile_pool(name="c2pool", bufs=2) as c2pool,
            tc.tile_pool(name="psum", bufs=8, space="PSUM") as psum,
        ):
            wsb = const.tile([128, (K + 1) * NBLK], F32, tag="wsb")
            nc.scalar.dma_start(out=wsb[:], in_=w_d)
            dwsb = const.tile([128, len(PE_BLKS) * K * 128], BF16, tag="dwsb")
            nc.scalar.dma_start(out=dwsb[:], in_=dw_d)
            zero = const.tile([128, CHUNK], BF16, tag="zero")
            nc.vector.memset(zero[:], 0.0)

            def wcol(k, blk):
                return wsb[:, k * NBLK + blk : k * NBLK + blk + 1]

            def dmat(pe_idx, k):
                c0 = (pe_idx * K + k) * 128
                return dwsb[:, c0 : c0 + 128]

            pending = []

            def flush_out(upto):
                while pending and pending[0][0] <= upto:
                    g, tile_ap = pending.pop(0)
                    g %= NGRP
                    nc.gpsimd.dma_start(
                        out=o_d[:, g * GRP * SHALF : (g + 1) * GRP * SHALF],
                        in_=tile_ap,
                    )

            xt = ot = None
            for blk_r in range(NBLK * nreps):
                blk = blk_r % NBLK
                g_r, j = divmod(blk_r, GRP)
                g = g_r % NGRP
                if j == 0:
                    flush_out(g_r - 2)
                    xt = xpool.tile([128, GRP * CW], BF16, tag="xt")
                    nc.sync.dma_start(
                        out=xt[:], in_=x_d[:, g * GRP * CW : (g + 1) * GRP * CW]
                    )
                    ot = opool.tile([128, GRP * SHALF], BF16, tag="ot")

                base = j * CW

                def tap(k, lo=0, n=SHALF):
                    return xt[:, base + k + lo : base + k + lo + n]

                osl = ot[:, j * SHALF : (j + 1) * SHALF]

                if blk in PE_IDX:
                    pi = PE_IDX[blk]
                    for c in range(SHALF // CHUNK):
                        ps = psum.tile([128, CHUNK], F32, tag="ps",
                                       name=f"ps_{blk_r}_{c}")
                        for k in range(K):
                            nc.tensor.matmul(
                                ps[:],
                                dmat(pi, k),
                                tap(k, c * CHUNK, CHUNK),
                                start=(k == 0),
                                stop=(k == K - 1),
                            )
                        oc = ot[:, j * SHALF + c * CHUNK : j * SHALF + (c + 1) * CHUNK]
                        if chunk_drain:
                            # alternate drain engine per chunk (ACT-biased
                            # 24/20 split across the 44 drains per pass)
                            dve = (4 * pi + c) % 11 < 5
                        else:
                            dve = blk in PE_DVE_DRAIN
                        if dve:
                            nc.vector.affine_then_add(
                                oc, ps[:], zero[:], 1.0, wcol(K, blk)
                            )
                        else:
                            nc.scalar.activation(
                                oc, ps[:], IDENT, bias=wcol(K, blk), scale=1.0
                            )
                else:
                    t3 = m3pool.tile([128, SHALF], BF16, tag="t3",
                                     name=f"t3_{blk_r}")
                    nc.scalar.activation(
                        t3[:], tap(3), IDENT, bias=wcol(K, blk), scale=wcol(3, blk)
                    )
                    t2 = c2pool.tile([128, SHALF], BF16, tag="t2",
                                     name=f"t2_{blk_r}")
                    nc.scalar.activation(
                        t2[:], tap(2), IDENT, bias=0.0, scale=wcol(2, blk)
                    )
                    s = d2pool.tile([128, SHALF], BF16, tag="s",
                                    name=f"s_{blk_r}")
                    nc.gpsimd.tensor_tensor(out=s[:], in0=t3[:], in1=t2[:], op=ADD)
                    d1 = d1pool.tile([128, SHALF], BF16, tag="d1",
                                     name=f"d1_{blk_r}")
                    nc.vector.affine_then_add(d1[:], tap(1), s[:], wcol(1, blk), 0.0)
                    nc.vector.scalar_tensor_tensor(
                        osl, tap(0), wcol(0, blk), d1[:], MULT, ADD
                    )

                if j == GRP - 1:
                    pending.append((g_r, ot[:]))
            flush_out(NGRP * nreps)

    nc.compile()
    _CACHE[key] = nc
    return nc


def _shard_inputs(x, weight, bias):
    x = np.asarray(x, dtype=np.float32)
    weight = np.asarray(weight, dtype=np.float32)
    bias = np.asarray(bias, dtype=np.float32)

    wr = weight[:, 0, :].reshape(NBLK, 128, K)          # [blk, p, k]
    wtab = np.empty((128, (K + 1) * NBLK), dtype=np.float32)
    wtab[:, : K * NBLK] = wr.transpose(1, 2, 0).reshape(128, K * NBLK)
    wtab[:, K * NBLK :] = bias.reshape(NBLK, 128).T

    dwt = np.zeros((128, len(PE_BLKS) * K * 128), dtype=np.float32)
    rng = np.arange(128)
    for pi, blk in enumerate(PE_BLKS):
        for k in range(K):
            dwt[rng, (pi * K + k) * 128 + rng] = wr[blk, :, k]
    dwt = _to_bf16(dwt)

    in_maps = []
    for core in range(NCORES):
        b, h = divmod(core, 2)
        s0 = h * SHALF
        xc = np.empty((NBLK, 128, CW), dtype=np.float32)
        xbt = x[b].T.reshape(NBLK, 128, S)  # [blk, p, s] view
        if s0 == 0:
            xc[:, :, :HIST] = 0.0
            xc[:, :, HIST:] = xbt[:, :, :SHALF]
        else:
            xc[:] = xbt[:, :, s0 - HIST : s0 + SHALF]
        xin = np.ascontiguousarray(xc.transpose(1, 0, 2)).reshape(128, NBLK * CW)
        in_maps.append(
            {"xin": _to_bf16(xin), "wtab": wtab, "dwt": dwt}
        )
    return in_maps


def _bf16_dtype():
    import ml_dtypes

    return np.dtype(ml_dtypes.bfloat16)


def _to_bf16(a):
    return np.asarray(a, dtype=np.float32).astype(_bf16_dtype())


def _run(x, weight, bias, trace=False):
    nc = _build_program()
    in_maps = _shard_inputs(x, weight, bias)
    res = run_bass_kernel_spmd(nc, in_maps, list(range(NCORES)), trace=trace)
    out = np.empty((B, S, D), dtype=np.float32)
    for core in range(NCORES):
        b, h = divmod(core, 2)
        o = np.asarray(res.results[core]["out"]).astype(np.float32)
        o = o.reshape(128, NBLK, SHALF).transpose(1, 0, 2).reshape(D, SHALF)
        out[b, h * SHALF : (h + 1) * SHALF, :] = o.T
    return out, res


def kernel(x, weight, bias):
    out, _ = _run(x, weight, bias, trace=False)
    return out
